# revision 1
# baseline (speedup 1.0000x reference)
"""TRN2 Bass kernel: TransformerXL-style MHA block (S=B=192, D=512, H=8).

Sharding: pure data-parallel over the batch axis across 8 NeuronCores
(24 batch elements per core). Host prep: batch-major transpose of the
input, transposes of the small weight matrices, and the sin/cos
positional table derived from inputs[0, :, 0]. Everything else
(double layernorm, QKV/R projections, rel-shift attention, output
projection) runs on device.

rel_shift trick: reference computes x_padded=(q,r+1) then reshapes.
Flattened, out[i,j] = padded_flat[S*i + j + S] where padded rows are
[0, x[i,0..r-1]] at pitch S+1.  We emit BD_raw via a matmul with an
extra leading zero column (N=193), store it contiguously in DRAM and
read it back through a strided access pattern -- no masking needed.
"""

import numpy as np
import ml_dtypes
from contextlib import ExitStack

import concourse.bass as bass
import concourse.bacc as bacc_mod
import concourse.mybir as mybir
import concourse.tile as tile
from concourse.bass_utils import run_bass_kernel_spmd
from concourse.masks import make_identity

S = 192
D = 512
H = 8
DH = 64
NCORES = 8
BL = 24            # batch elements per core
GB = 2             # batch elements per inner group
NG = BL // GB      # 12 groups
TOK = GB * S       # 384 tokens per group
NT = TOK // 128    # 3 token tiles per group
EPS = 1e-5
SCALE = 1.0 / (DH ** 0.5)
PAD = S + 1        # 193: rel-shift row pitch

f32 = mybir.dt.float32
f32r = mybir.dt.float32r
bf16 = mybir.dt.bfloat16
AF = mybir.ActivationFunctionType
OP = mybir.AluOpType


def _r(ap, dt):
    return ap.bitcast(dt)


def build(identity_ln: bool) -> bass.Bass:
    nc = bacc_mod.Bacc()

    x = nc.dram_tensor("x", [BL * S, D], f32, kind="ExternalInput")
    wqkvT = nc.dram_tensor("wqkvT", [D, 3 * D], f32, kind="ExternalInput")
    wrT = nc.dram_tensor("wrT", [D, D], bf16, kind="ExternalInput")
    woT = nc.dram_tensor("woT", [D, D], f32, kind="ExternalInput")
    pembT = nc.dram_tensor("pembT", [D, S], bf16, kind="ExternalInput")
    rwbs = nc.dram_tensor("rwbs", [D], f32, kind="ExternalInput")  # 0.125*r_w_bias
    rrb = nc.dram_tensor("rrb", [D], f32, kind="ExternalInput")
    if not identity_ln:
        g1d = nc.dram_tensor("g1", [D], f32, kind="ExternalInput")
        b1d = nc.dram_tensor("b1", [D], f32, kind="ExternalInput")
        g2d = nc.dram_tensor("g2", [D], f32, kind="ExternalInput")
        b2d = nc.dram_tensor("b2", [D], f32, kind="ExternalInput")
    out = nc.dram_tensor("out", [BL * S, D], f32, kind="ExternalOutput")

    with TileKernel(nc) as tk:
        tk.run(locals())
    nc.finalize()
    return nc


class TileKernel:
    def __init__(self, nc):
        self.nc = nc
        self.ctx = ExitStack()

    def __enter__(self):
        self.ctx.__enter__()
        self.tc = self.ctx.enter_context(tile.TileContext(self.nc))
        return self

    def __exit__(self, *a):
        return self.ctx.__exit__(*a)

    def pool(self, name, bufs, space="SBUF"):
        return self.ctx.enter_context(
            self.tc.tile_pool(name=name, bufs=bufs, space=space)
        )

    def run(self, env):
        nc = self.nc
        x, out = env["x"], env["out"]
        wqkvT, wrT, woT, pembT = env["wqkvT"], env["wrT"], env["woT"], env["pembT"]
        rwbs, rrb = env["rwbs"], env["rrb"]
        identity_ln = env["identity_ln"]

        consts = self.pool("consts", 1)
        xio = self.pool("xio", 4)
        hp = self.pool("h", 4)
        stats = self.pool("stats", 12)
        htp = self.pool("hT", 2)
        qkp = self.pool("qk", 2)
        vp = self.pool("v", 2)
        atp = self.pool("attnT", 2)
        bdw = self.pool("bdw", 3)
        bdr = self.pool("bdr", 3)
        ep = self.pool("ep", 3)
        ptp = self.pool("pt", 3)
        op_ = self.pool("oev", 3)
        psA = self.pool("psA", 2, space="PSUM")
        psB = self.pool("psB", 4, space="PSUM")
        psT = self.pool("psT", 2, space="PSUM")
        dpool = self.pool("bddram", 6, space="DRAM")

        # ---- constants ----
        idf = consts.tile([128, 128], f32)
        make_identity(nc, idf)
        idb = consts.tile([128, 128], bf16)
        make_identity(nc, idb)
        eps_t = consts.tile([128, 1], f32)
        nc.vector.memset(eps_t, EPS)

        wq_sb = consts.tile([128, 4, 3 * D], f32r)
        wo_sb = consts.tile([128, 4, D], f32r)
        wr_sb = consts.tile([128, 4, D], bf16)
        pe_sb = consts.tile([128, 4, S], bf16)
        nc.sync.dma_start(
            out=wq_sb, in_=_r(wqkvT[:].rearrange("(c p) e -> p c e", c=4), f32r)
        )
        nc.sync.dma_start(
            out=wo_sb, in_=_r(woT[:].rearrange("(c p) e -> p c e", c=4), f32r)
        )
        nc.sync.dma_start(out=wr_sb, in_=wrT[:].rearrange("(c p) e -> p c e", c=4))
        nc.sync.dma_start(out=pe_sb, in_=pembT[:].rearrange("(c p) e -> p c e", c=4))

        rw_sb = consts.tile([128, 4], f32)
        rr_sb = consts.tile([128, 4], f32)
        nc.sync.dma_start(out=rw_sb, in_=rwbs[:].rearrange("(c p) -> p c", c=4))
        nc.sync.dma_start(out=rr_sb, in_=rrb[:].rearrange("(c p) -> p c", c=4))

        if not identity_ln:
            gtiles = []
            for name in ("g1", "b1", "g2", "b2"):
                t = consts.tile([128, D], f32, tag=f"ln_{name}")
                src = env[name + "d"][:]
                bcast = bass.AP(
                    tensor=src.tensor, offset=src.offset, ap=[[0, 128], [1, D]]
                )
                nc.sync.dma_start(out=t, in_=bcast)
                gtiles.append(t)
            g1t, b1t, g2t, b2t = gtiles

        # rhk_ext: (128 part, head-chunk 4, 1+S) bf16, col 0 zero, rest 0.125*rhk
        rhk_sb = consts.tile([128, 4, PAD], bf16)
        nc.vector.memset(rhk_sb, 0.0)
        for c in range(4):
            pr = psA.tile([128, S], f32, tag="ps_big")
            for kc in range(4):
                nc.tensor.matmul(
                    pr,
                    wr_sb[:, kc, c * 128:(c + 1) * 128],
                    pe_sb[:, kc, :],
                    start=(kc == 0),
                    stop=(kc == 3),
                )
            nc.scalar.activation(
                out=rhk_sb[:, c, 1:PAD], in_=pr, func=AF.Copy, scale=SCALE
            )

        x2 = x[:]
        out2 = out[:]

        for g in range(NG):
            # ---- layer norms + transpose to hT (d-major) ----
            hT = htp.tile([128, 4, TOK], f32r)
            for t in range(NT):
                r0 = g * TOK + t * 128
                xt = xio.tile([128, D], f32)
                nc.sync.dma_start(out=xt, in_=x2[r0:r0 + 128, :])
                ht = hp.tile([128, D], f32)
                cur = xt
                for ln_i in range(2):
                    st = stats.tile([128, 6], f32, tag="bn6")
                    mv = stats.tile([128, 2], f32, tag="mv")
                    nc.vector.bn_stats(out=st, in_=cur)
                    nc.vector.bn_aggr(out=mv, in_=st)
                    rstd = stats.tile([128, 1], f32, tag="rstd")
                    nc.scalar.activation(
                        out=rstd, in_=mv[:, 1:2], func=AF.Sqrt, bias=eps_t, scale=1.0
                    )
                    nc.vector.reciprocal(out=rstd, in_=rstd)
                    if identity_ln:
                        nc.vector.tensor_scalar(
                            out=ht,
                            in0=cur,
                            scalar1=mv[:, 0:1],
                            scalar2=rstd,
                            op0=OP.subtract,
                            op1=OP.mult,
                        )
                    else:
                        gt = g1t if ln_i == 0 else g2t
                        bt = b1t if ln_i == 0 else b2t
                        nc.vector.scalar_tensor_tensor(
                            out=ht,
                            in0=cur,
                            scalar=mv[:, 0:1],
                            in1=gt,
                            op0=OP.subtract,
                            op1=OP.mult,
                        )  # (x - mean) * g
                        nc.vector.tensor_scalar_mul(out=ht, in0=ht, scalar1=rstd)
                        nc.vector.tensor_add(out=ht, in0=ht, in1=bt)
                    cur = ht
                for c in range(4):
                    pt_ = psA.tile([128, 128], f32, tag="ps_big")
                    nc.tensor.transpose(pt_, ht[:, c * 128:(c + 1) * 128], idf)
                    nc.scalar.activation(
                        out=hT[:, c, t * 128:(t + 1) * 128], in_=pt_, func=AF.Copy
                    )

            # ---- QKV projections ----
            qT = qkp.tile([128, 4, TOK], bf16, tag="qT")
            rrT = qkp.tile([128, 4, TOK], bf16, tag="rrT")
            kT = qkp.tile([128, 4, TOK], bf16, tag="kT")
            for c in range(8):
                pq = psA.tile([128, TOK], f32, tag="ps_big")
                for kc in range(4):
                    nc.tensor.matmul(
                        pq,
                        wq_sb[:, kc, c * 128:(c + 1) * 128],
                        hT[:, kc, :],
                        start=(kc == 0),
                        stop=(kc == 3),
                    )
                if c < 4:
                    nc.vector.tensor_scalar(
                        out=qT[:, c, :],
                        in0=pq,
                        scalar1=SCALE,
                        scalar2=rw_sb[:, c:c + 1],
                        op0=OP.mult,
                        op1=OP.add,
                    )
                    nc.vector.tensor_scalar_add(
                        out=rrT[:, c, :], in0=pq, scalar1=rr_sb[:, c:c + 1]
                    )
                else:
                    nc.scalar.activation(
                        out=kT[:, c - 4, :], in_=pq, func=AF.Copy
                    )
            # v in natural (token, e) layout, split per batch element so PV
            # contraction chunks align with the P^T partition split (128+64)
            va = vp.tile([128, GB, D], bf16, tag="va")  # tokens j in [0,128)
            vb = vp.tile([64, GB, D], bf16, tag="vb")   # tokens j in [128,192)
            for bl in range(GB):
                pv = psA.tile([128, D], f32, tag="ps_big")
                for kc in range(4):
                    nc.tensor.matmul(
                        pv,
                        hT[:, kc, bl * S:bl * S + 128],
                        wq_sb[:, kc, 2 * D:3 * D],
                        start=(kc == 0),
                        stop=(kc == 3),
                    )
                nc.scalar.activation(out=va[:, bl, :], in_=pv, func=AF.Copy)
                pv2 = psA.tile([64, D], f32, tag="ps_big")
                for kc in range(4):
                    nc.tensor.matmul(
                        pv2,
                        hT[:, kc, bl * S + 128:(bl + 1) * S],
                        wq_sb[:, kc, 2 * D:3 * D],
                        start=(kc == 0),
                        stop=(kc == 3),
                    )
                nc.scalar.activation(out=vb[:, bl, :], in_=pv2, func=AF.Copy)

            # ---- attention per (b, head) ----
            attnT = atp.tile([128, 4, TOK], f32r)
            for bl in range(GB):
                tok0 = bl * S
                for h in range(H):
                    po = (h % 2) * 64
                    c4 = h // 2
                    qs = qT[po:po + 64, c4, tok0:tok0 + S]
                    rs = rrT[po:po + 64, c4, tok0:tok0 + S]
                    ks = kT[po:po + 64, c4, tok0:tok0 + S]
                    rh = rhk_sb[po:po + 64, c4, :]

                    # BD_raw with leading zero col, -> DRAM
                    bdt = dpool.tile([S, PAD], bf16)
                    pb0 = psB.tile([128, PAD], f32, tag="ps_att")
                    nc.tensor.matmul(pb0, rs[:, 0:128], rh, start=True, stop=True)
                    b0 = bdw.tile([128, PAD], bf16, tag="bd0")
                    nc.scalar.activation(out=b0, in_=pb0, func=AF.Copy)
                    nc.sync.dma_start(out=bdt[0:128, :], in_=b0)
                    pb1 = psB.tile([64, PAD], f32, tag="ps_att")
                    nc.tensor.matmul(pb1, rs[:, 128:S], rh, start=True, stop=True)
                    b1_ = bdw.tile([64, PAD], bf16, tag="bd1")
                    nc.scalar.activation(out=b1_, in_=pb1, func=AF.Copy)
                    nc.sync.dma_start(out=bdt[128:S, :], in_=b1_)

                    # AC
                    ps0 = psB.tile([128, S], f32, tag="ps_att")
                    nc.tensor.matmul(ps0, qs[:, 0:128], ks, start=True, stop=True)
                    ps1 = psB.tile([64, S], f32, tag="ps_att")
                    nc.tensor.matmul(ps1, qs[:, 128:S], ks, start=True, stop=True)

                    # shifted BD read
                    base = bdt[:, :]
                    sh0 = bass.AP(
                        tensor=base.tensor, offset=base.offset + S,
                        ap=[[S, 128], [1, S]],
                    )
                    sh1 = bass.AP(
                        tensor=base.tensor, offset=base.offset + S + 128 * S,
                        ap=[[S, 64], [1, S]],
                    )
                    bd0 = bdr.tile([128, S], bf16, tag="bdr0")
                    nc.sync.dma_start(out=bd0, in_=sh0)
                    bd1 = bdr.tile([64, S], bf16, tag="bdr1")
                    nc.sync.dma_start(out=bd1, in_=sh1)

                    # S = AC + BD ; E = exp(S), denom via accum
                    s0 = ep.tile([128, S], bf16, tag="s0")
                    nc.vector.tensor_add(out=s0, in0=ps0, in1=bd0)
                    s1 = ep.tile([64, S], bf16, tag="s1")
                    nc.vector.tensor_add(out=s1, in0=ps1, in1=bd1)
                    e0 = ep.tile([128, S], bf16, tag="e0")
                    den0 = stats.tile([128, 1], f32, tag="den0")
                    nc.scalar.activation(
                        out=e0, in_=s0, func=AF.Exp, accum_out=den0
                    )
                    e1 = ep.tile([64, S], bf16, tag="e1")
                    den1 = stats.tile([64, 1], f32, tag="den1")
                    nc.scalar.activation(
                        out=e1, in_=s1, func=AF.Exp, accum_out=den1
                    )
                    rd0 = stats.tile([128, 1], f32, tag="rd0")
                    nc.vector.reciprocal(out=rd0, in_=den0)
                    rd1 = stats.tile([64, 1], f32, tag="rd1")
                    nc.vector.reciprocal(out=rd1, in_=den1)
                    p0 = ep.tile([128, S], bf16, tag="p0")
                    nc.gpsimd.tensor_scalar_mul(out=p0, in0=e0, scalar1=rd0)
                    p1 = ep.tile([64, S], bf16, tag="p1")
                    nc.gpsimd.tensor_scalar_mul(out=p1, in0=e1, scalar1=rd1)

                    # P^T via PE transposes
                    pt0 = psT.tile([128, S], bf16, tag="ps_att_t")
                    nc.tensor.transpose(pt0[:, 0:128], p0[:, 0:128], idb)
                    nc.tensor.transpose(pt0[:, 128:S], p1[:, 0:128], idb[0:64, 0:64])
                    pt1 = psT.tile([64, S], bf16, tag="ps_att_t")
                    nc.tensor.transpose(pt1[:, 0:128], p0[:, 128:S], idb)
                    nc.tensor.transpose(pt1[:, 128:S], p1[:, 128:S], idb[0:64, 0:64])
                    pts0 = ptp.tile([128, S], bf16, tag="pts0")
                    nc.vector.tensor_copy(out=pts0, in_=pt0)
                    pts1 = ptp.tile([64, S], bf16, tag="pts1")
                    nc.vector.tensor_copy(out=pts1, in_=pt1)

                    # PV: lhsT = v chunks (tokens of bl), rhs = P^T
                    # psum allocated full-height; write rows [po, po+64) so the
                    # eviction to attnT keeps partition alignment
                    ppv = psB.tile([128, S], f32, tag="ps_att")
                    hc = slice(h * DH, (h + 1) * DH)
                    nc.tensor.matmul(
                        ppv[po:po + 64, :], va[:, bl, hc], pts0,
                        start=True, stop=False,
                    )
                    nc.tensor.matmul(
                        ppv[po:po + 64, :], vb[:, bl, hc], pts1,
                        start=False, stop=True,
                    )
                    nc.vector.tensor_copy(
                        out=attnT[po:po + 64, c4, tok0:tok0 + S],
                        in_=ppv[po:po + 64, :],
                    )

            # ---- output projection ----
            for t in range(NT):
                pw = psA.tile([128, D], f32, tag="ps_big")
                for kc in range(4):
                    nc.tensor.matmul(
                        pw,
                        attnT[:, kc, t * 128:(t + 1) * 128],
                        wo_sb[:, kc, :],
                        start=(kc == 0),
                        stop=(kc == 3),
                    )
                ot = op_.tile([128, D], f32)
                nc.scalar.activation(out=ot, in_=pw, func=AF.Copy)
                r0 = g * TOK + t * 128
                nc.sync.dma_start(out=out2[r0:r0 + 128, :], in_=ot)


_CACHE = {}


def _get_nc(identity_ln):
    key = bool(identity_ln)
    if key not in _CACHE:
        _CACHE[key] = build(key)
    return _CACHE[key]


LAST_RESULT = None


def _is_identity_ln(ln1_g, ln1_b, ln2_g, ln2_b):
    return bool(
        np.all(np.asarray(ln1_g) == 1.0)
        and np.all(np.asarray(ln1_b) == 0.0)
        and np.all(np.asarray(ln2_g) == 1.0)
        and np.all(np.asarray(ln2_b) == 0.0)
    )


def prep_in_maps(d):
    inputs = np.asarray(d["inputs"], dtype=np.float32)
    x_bmaj = np.ascontiguousarray(inputs.transpose(1, 0, 2))  # (B, S, D)

    pos_seq = np.asarray(inputs[0, :, 0], dtype=np.float32)
    inv_freq = (
        1.0 / (10000.0 ** (np.arange(0, D, 2, dtype=np.float32) / np.float32(D)))
    ).astype(np.float32)
    sin_inp = pos_seq[:, None].astype(np.float32) * inv_freq[None, :]
    pos_emb = np.concatenate([np.sin(sin_inp), np.cos(sin_inp)], axis=-1)
    pembT = np.ascontiguousarray(pos_emb.T.astype(ml_dtypes.bfloat16))  # (D, S)

    wqkvT = np.ascontiguousarray(np.asarray(d["W_qkv"], np.float32).T)
    wrT = np.ascontiguousarray(np.asarray(d["W_r"], np.float32).T.astype(ml_dtypes.bfloat16))
    woT = np.ascontiguousarray(np.asarray(d["W_o"], np.float32).T)
    rwbs = np.ascontiguousarray(
        np.asarray(d["r_w_bias"], np.float32).reshape(D) * np.float32(SCALE)
    )
    rrb = np.ascontiguousarray(np.asarray(d["r_r_bias"], np.float32).reshape(D))

    identity_ln = _is_identity_ln(d["ln1_g"], d["ln1_b"], d["ln2_g"], d["ln2_b"])
    shared = {
        "wqkvT": wqkvT, "wrT": wrT, "woT": woT, "pembT": pembT,
        "rwbs": rwbs, "rrb": rrb,
    }
    if not identity_ln:
        shared.update({
            "g1": np.asarray(d["ln1_g"], np.float32),
            "b1": np.asarray(d["ln1_b"], np.float32),
            "g2": np.asarray(d["ln2_g"], np.float32),
            "b2": np.asarray(d["ln2_b"], np.float32),
        })
    in_maps = []
    for i in range(NCORES):
        m = dict(shared)
        m["x"] = np.ascontiguousarray(
            x_bmaj[i * BL:(i + 1) * BL].reshape(BL * S, D)
        )
        in_maps.append(m)
    return in_maps


def kernel(inputs, ln1_g, ln1_b, ln2_g, ln2_b, W_qkv, W_r, W_o, r_w_bias, r_r_bias):
    global LAST_RESULT
    d = dict(
        inputs=inputs, ln1_g=ln1_g, ln1_b=ln1_b, ln2_g=ln2_g, ln2_b=ln2_b,
        W_qkv=W_qkv, W_r=W_r, W_o=W_o, r_w_bias=r_w_bias, r_r_bias=r_r_bias,
    )
    identity_ln = _is_identity_ln(ln1_g, ln1_b, ln2_g, ln2_b)
    nc = _get_nc(identity_ln)
    in_maps = prep_in_maps(d)

    res = run_bass_kernel_spmd(nc, in_maps, core_ids=list(range(NCORES)))
    LAST_RESULT = res
    outs = [r["out"].reshape(BL, S, D) for r in res.results]
    full = np.concatenate(outs, axis=0)          # (B, S, D)
    return np.ascontiguousarray(full.transpose(1, 0, 2))  # (S, B, D)

